# revision 1
# baseline (speedup 1.0000x reference)
"""Multi-head attention (bs=4, seq=2048, hidden=1024, 16 heads) on 8 trn2 cores.

Sharding: core = (batch b, head-group g) with 4 batches x 2 groups of 8 heads.
Each core computes QKV projections for its head slice, causal+padded softmax
attention, and a partial output projection; the host sums the two partial
outputs per batch and adds o_b.

Device layout (per core):
  xT   [1024, 2048]  hidden[b]^T           (host-transposed)
  wqT/wkT/wvT [1024, 512]  W[rows r]^T     (host-transposed slices)
  woT  [512, 1024]   o_w[:, r]^T
  qT/kT = W^T-projections in [o, s] layout; v in [s, o] layout with a ones
  column per head (augmented-V) so softmax denominators accumulate in the
  same PSUM bank as the attention output.
Scores are computed transposed [sk, sq] so the softmax sum is a matmul
reduction; exp runs on ScalarE with the padding mask as a per-partition bias;
the causal triangle is zeroed multiplicatively on VectorE after exp.
"""
import os
import sys

for _p in ("/opt/trn_rl_repo",):
    if _p not in sys.path:
        sys.path.insert(0, _p)

import numpy as np

HID = 1024
HEADS = 16
D = 64
BS = 4
SEQ = 2048
NCORES = 8
HG = 2            # head groups (tensor-parallel axis)
HPG = HEADS // HG  # 8 heads per core
OG = HPG * D       # 512 projection dims per core
KC = HID // 128    # 8 hidden chunks
TQ = 4             # sq tiles
TW = SEQ // TQ     # 512 queries per tile
SC = SEQ // 128    # 16 s chunks
SCALE = 1.0 / np.sqrt(D)

_compiled = None


def _chunks_for_tile(t):
    """(sk_chunk, col_offset, width) list for sq-tile t (causal structure)."""
    out = [(c, 0, TW) for c in range(4 * t)]
    for i in range(4):
        out.append((4 * t + i, 128 * i, TW - 128 * i))
    return out


def _build():
    import concourse.tile as tile
    from concourse import bacc, mybir

    F32 = mybir.dt.float32
    F32R = mybir.dt.float32r
    BF16 = mybir.dt.bfloat16
    AF = mybir.ActivationFunctionType
    Alu = mybir.AluOpType

    nc = bacc.Bacc("TRN2", target_bir_lowering=False, debug=False,
                   num_devices=NCORES)

    xT_d = nc.dram_tensor("xT", [HID, SEQ], F32R, kind="ExternalInput").ap()
    wqT_d = nc.dram_tensor("wqT", [HID, OG], F32R, kind="ExternalInput").ap()
    wkT_d = nc.dram_tensor("wkT", [HID, OG], F32R, kind="ExternalInput").ap()
    wvT_d = nc.dram_tensor("wvT", [HID, OG], F32R, kind="ExternalInput").ap()
    woT_d = nc.dram_tensor("woT", [OG, HID], F32R, kind="ExternalInput").ap()
    qb_d = nc.dram_tensor("qb", [128, 4], F32, kind="ExternalInput").ap()
    kb_d = nc.dram_tensor("kb", [128, 4], F32, kind="ExternalInput").ap()
    vb_d = nc.dram_tensor("vb", [1, OG], F32R, kind="ExternalInput").ap()
    kmask_d = nc.dram_tensor("kmask", [128, SC], F32, kind="ExternalInput").ap()
    out_d = nc.dram_tensor("out", [SEQ, HID], F32, kind="ExternalOutput").ap()

    with tile.TileContext(nc) as tc:
        with tc.tile_pool(name="const", bufs=1) as cp, \
             tc.tile_pool(name="qT", bufs=1) as qTp, \
             tc.tile_pool(name="kT", bufs=1) as kTp, \
             tc.tile_pool(name="v", bufs=1) as vp, \
             tc.tile_pool(name="attnT", bufs=1) as aTp:

            ones_f = cp.tile([128, 128], F32, tag="ones_f")
            nc.gpsimd.memset(ones_f[:, :], 1.0)
            ones = cp.tile([128, 128], F32R, tag="ones")
            nc.scalar.copy(ones[:, :], ones_f[:, :])
            # tri01[p, j] = 1 if j >= p else 0  (keep keys <= query)
            tri01_f = cp.tile([128, 128], F32, tag="tri01_f")
            nc.gpsimd.affine_select(tri01_f[:, :], ones_f[:, :],
                                    pattern=[[1, 128]],
                                    compare_op=Alu.is_ge, fill=0.0,
                                    base=0, channel_multiplier=-1)
            tri01 = cp.tile([128, 128], BF16, tag="tri01")
            nc.scalar.copy(tri01[:, :], tri01_f[:, :])
            qb_s = cp.tile([128, 4], F32, tag="qb")
            nc.sync.dma_start(qb_s[:, :], qb_d[:, :])
            kb_s = cp.tile([128, 4], F32, tag="kb")
            nc.sync.dma_start(kb_s[:, :], kb_d[:, :])
            vb_s = cp.tile([1, OG], F32R, tag="vb")
            nc.sync.dma_start(vb_s[:, :], vb_d[:, :])
            kmask_s = cp.tile([128, SC], F32, tag="km")
            nc.sync.dma_start(kmask_s[:, :], kmask_d[:, :])

            qT_t = [qTp.tile([128, SEQ], F32R, tag=f"qT{i}", name=f"qT{i}") for i in range(4)]
            kT_t = [kTp.tile([128, SEQ], F32R, tag=f"kT{i}", name=f"kT{i}") for i in range(4)]
            v_t = [vp.tile([128, HPG * 65], BF16, tag=f"v{i}", name=f"v{i}") for i in range(SC)]

            # ---------------- phase 1: projections (2 seq halves) ---------
            HSEQ = SEQ // 2
            for half in range(2):
                hs = half * HSEQ
                with tc.tile_pool(name=f"xT{half}", bufs=1) as xp:
                    xT_t = []
                    for kc in range(KC):
                        xt = xp.tile([128, HSEQ], F32R, tag=f"xT{kc}",
                                     name=f"xTh{half}_{kc}")
                        nc.sync.dma_start(
                            xt[:, :], xT_d[kc * 128:(kc + 1) * 128,
                                           hs:hs + HSEQ])
                        xT_t.append(xt)

                    with tc.tile_pool(name=f"wqk{half}", bufs=1) as wp, \
                         tc.tile_pool(name=f"ps1_{half}", bufs=6,
                                      space="PSUM") as ps1:
                        wq_t, wk_t = [], []
                        for kc in range(KC):
                            wq = wp.tile([128, OG], F32R, tag=f"wq{kc}",
                                         name=f"wqh{half}_{kc}")
                            nc.sync.dma_start(
                                wq[:, :], wqT_d[kc * 128:(kc + 1) * 128, :])
                            wq_t.append(wq)
                            wk = wp.tile([128, OG], F32R, tag=f"wk{kc}",
                                         name=f"wkh{half}_{kc}")
                            nc.sync.dma_start(
                                wk[:, :], wkT_d[kc * 128:(kc + 1) * 128, :])
                            wk_t.append(wk)

                        for w_t, o_t, bias in ((wq_t, qT_t, qb_s),
                                               (wk_t, kT_t, kb_s)):
                            for oc in range(4):
                                pts = [ps1.tile([128, TW], F32, tag="p1",
                                                name="p1")
                                       for _ in range(HSEQ // TW)]
                                for kc in range(KC):
                                    for t in range(HSEQ // TW):
                                        nc.tensor.matmul(
                                            pts[t][:, :],
                                            w_t[kc][:, oc * 128:(oc + 1) * 128],
                                            xT_t[kc][:, t * TW:(t + 1) * TW],
                                            start=(kc == 0),
                                            stop=(kc == KC - 1))
                                for t in range(HSEQ // TW):
                                    nc.scalar.activation(
                                        o_t[oc][:, hs + t * TW:hs + (t + 1) * TW],
                                        pts[t][:, :], AF.Identity,
                                        bias=bias[:, oc:oc + 1], scale=1.0)

                    with tc.tile_pool(name=f"wv{half}", bufs=1) as wvp, \
                         tc.tile_pool(name=f"ps1b{half}", bufs=6,
                                      space="PSUM") as ps1b:
                        wv_t = []
                        for kc in range(KC):
                            wv = wvp.tile([128, OG], F32R, tag=f"wv{kc}",
                                          name=f"wvh{half}_{kc}")
                            nc.sync.dma_start(
                                wv[:, :], wvT_d[kc * 128:(kc + 1) * 128, :])
                            wv_t.append(wv)

                        for sc in range(SC // 2):
                            scg = half * (SC // 2) + sc
                            pv = ps1b.tile([128, OG], F32, tag="pv", name="pv")
                            for kc in range(KC):
                                nc.tensor.matmul(
                                    pv[:, :],
                                    xT_t[kc][:, sc * 128:(sc + 1) * 128],
                                    wv_t[kc][:, :],
                                    start=(kc == 0), stop=False)
                            # + v_b via ones-outer-product
                            nc.tensor.matmul(pv[:, :], ones[0:1, :],
                                             vb_s[0:1, :],
                                             start=False, stop=True)
                            src = pv.rearrange("p (h c) -> p h c", c=64)
                            dst = v_t[scg].rearrange("p (h c) -> p h c", c=65)
                            nc.scalar.activation(dst[:, :, 0:64], src[:, :, :],
                                                 AF.Copy)
                            nc.scalar.activation(
                                dst[:, :, 64:65],
                                ones_f[:, 0:HPG].unsqueeze(2),
                                AF.Copy)

            # ---------------- phase 2: attention ----------------
            # sq windows of 1024; scores land in 2-bank PSUM tiles so exp
            # runs as one wide ACT op per sk-chunk.
            attnT_t = [aTp.tile([128, SEQ], F32R, tag=f"aT{i}", name=f"aT{i}") for i in range(4)]
            W = 1024
            with tc.tile_pool(name="ph2", bufs=1) as p2, \
                 tc.tile_pool(name="ps2", bufs=1, space="PSUM") as ps2:
                for tw in range(SEQ // W):
                    chunks = [(c, 0, W) for c in range(8 * tw)]
                    chunks += [(8 * tw + i, 128 * i, W - 128 * i)
                               for i in range(8)]
                    # last chunk writing each 512-half of the window
                    last0 = max(i for i, (_, off, _) in enumerate(chunks)
                                if off < 512)
                    last1 = len(chunks) - 1
                    for h in range(HPG):
                        hb = (h % 2) * 64
                        hc = h // 2
                        kslice = kT_t[hc]
                        at0 = ps2.tile([128, 512], F32, tag="at", bufs=3,
                                       name="at0")
                        at1 = ps2.tile([128, 512], F32, tag="at", bufs=3,
                                       name="at1")
                        ats = (at0, at1)
                        for idx, (c, off, w) in enumerate(chunks):
                            sp = ps2.tile([128, W], F32, tag="sc", bufs=2)
                            for lo, hi in ((off, 512), (max(off, 512), W)):
                                if lo >= hi:
                                    continue
                                nc.tensor.matmul(
                                    sp[:, lo:hi],
                                    kslice[hb:hb + 64, c * 128:(c + 1) * 128],
                                    qT_t[hc][hb:hb + 64, tw * W + lo:tw * W + hi],
                                    start=True, stop=True)
                            et = p2.tile([128, W], BF16, tag="E", bufs=4)
                            nc.scalar.activation(et[:, :w], sp[:, off:off + w],
                                                 AF.Exp,
                                                 bias=kmask_s[:, c:c + 1],
                                                 scale=SCALE)
                            if off or c == 8 * tw:  # diagonal chunk
                                nc.vector.tensor_mul(et[:, 0:128], et[:, 0:128],
                                                     tri01[:, :])
                            for half in range(2):
                                lo = max(off, half * 512)
                                hi = (half + 1) * 512
                                if lo >= hi:
                                    continue
                                nc.tensor.matmul(
                                    ats[half][0:65, lo - half * 512:512],
                                    v_t[c][:, h * 65:(h + 1) * 65],
                                    et[:, lo - off:hi - off],
                                    start=(idx == 0),
                                    stop=(idx == (last0, last1)[half]))
                        for half in range(2):
                            at = ats[half]
                            recip = p2.tile([128, 512], F32R, tag="rc", bufs=2)
                            with nc.allow_low_precision("fp32r recip"):
                                nc.vector.reciprocal(recip[64:65, :],
                                                     at[64:65, :])
                            # broadcast recip row to 64 rows via K=1 matmul
                            dps = ps2.tile([128, 512], F32, tag="dps", bufs=1)
                            nc.tensor.matmul(dps[0:64, :], ones[64:65, 0:64],
                                             recip[64:65, :],
                                             start=True, stop=True)
                            div = p2.tile([128, 512], F32, tag="dv", bufs=2)
                            nc.vector.tensor_copy(div[0:64, :], dps[0:64, :])
                            tcols = slice(tw * W + half * 512,
                                          tw * W + (half + 1) * 512)
                            if h % 2 == 0:
                                nc.vector.tensor_mul(attnT_t[hc][0:64, tcols],
                                                     at[0:64, :], div[0:64, :])
                            else:
                                tmp = p2.tile([64, 512], F32R, tag="tm",
                                              bufs=2)
                                nc.vector.tensor_mul(tmp[:, :], at[0:64, :],
                                                     div[0:64, :])
                                nc.sync.dma_start(attnT_t[hc][64:128, tcols],
                                                  tmp[:, :])

            # ---------------- phase 3: output projection ----------------
            with tc.tile_pool(name="ph3", bufs=1) as p3, \
                 tc.tile_pool(name="ps3", bufs=4, space="PSUM") as ps3:
                wo_t = []
                for kc in range(4):
                    wo = p3.tile([128, HID], F32R, tag=f"wo{kc}")
                    nc.sync.dma_start(wo[:, :], woT_d[kc * 128:(kc + 1) * 128, :])
                    wo_t.append(wo)
                for sc in range(SC):
                    ot = p3.tile([128, HID], F32, tag="ou", bufs=3)
                    for n in range(2):
                        po = ps3.tile([128, 512], F32, tag="p3")
                        for kc in range(4):
                            nc.tensor.matmul(
                                po[:, :],
                                attnT_t[kc][:, sc * 128:(sc + 1) * 128],
                                wo_t[kc][:, n * 512:(n + 1) * 512],
                                start=(kc == 0), stop=(kc == 3))
                        nc.vector.tensor_copy(ot[:, n * 512:(n + 1) * 512],
                                              po[:, :])
                    nc.sync.dma_start(out_d[sc * 128:(sc + 1) * 128, :], ot[:, :])

    nc.compile()
    return nc


def kernel(hidden_states, causal_mask, padding_mask,
           q_w, q_b, k_w, k_b, v_w, v_b, o_w, o_b):
    global _compiled
    from concourse.bass_utils import run_bass_kernel_spmd

    hidden_states = np.asarray(hidden_states, dtype=np.float32)
    padding_mask = np.asarray(padding_mask)
    q_w = np.asarray(q_w, dtype=np.float32)
    k_w = np.asarray(k_w, dtype=np.float32)
    v_w = np.asarray(v_w, dtype=np.float32)
    o_w = np.asarray(o_w, dtype=np.float32)
    q_b = np.asarray(q_b, dtype=np.float32)
    k_b = np.asarray(k_b, dtype=np.float32)
    v_b = np.asarray(v_b, dtype=np.float32)
    o_b = np.asarray(o_b, dtype=np.float32)

    if _compiled is None:
        _compiled = _build()
    nc = _compiled

    in_maps = []
    for b in range(BS):
        xT = np.ascontiguousarray(hidden_states[b].T)
        kmask = np.where(padding_mask[b], np.float32(-30000.0),
                         np.float32(0.0)).astype(np.float32)
        kmask2 = np.ascontiguousarray(kmask.reshape(SC, 128).T)
        for g in range(HG):
            r = slice(g * OG, (g + 1) * OG)
            in_maps.append({
                "xT": xT,
                "wqT": np.ascontiguousarray(q_w[r].T),
                "wkT": np.ascontiguousarray(k_w[r].T),
                "wvT": np.ascontiguousarray(v_w[r].T),
                "woT": np.ascontiguousarray(o_w[:, r].T),
                "qb": np.ascontiguousarray(q_b[r].reshape(4, 128).T),
                "kb": np.ascontiguousarray(k_b[r].reshape(4, 128).T),
                "vb": np.ascontiguousarray(v_b[r].reshape(1, OG)),
                "kmask": kmask2,
            })

    trace = os.environ.get("KERNEL_TRACE") == "1"
    res = run_bass_kernel_spmd(nc, in_maps, core_ids=list(range(NCORES)),
                               trace=trace)
    if trace and res.exec_time_ns is not None:
        print(f"HW exec time: {res.exec_time_ns} ns")
        if res.instructions_and_trace:
            print(f"trace: {res.instructions_and_trace[1]}")

    out = np.empty((BS, SEQ, HID), dtype=np.float32)
    for b in range(BS):
        out[b] = (res.results[2 * b]["out"] + res.results[2 * b + 1]["out"]
                  + o_b[None, :])
    return out



# revision 2
# speedup vs baseline: 1.2479x; 1.2479x over previous
"""Multi-head attention (bs=4, seq=2048, hidden=1024, 16 heads) on 8 trn2 cores.

Sharding: core = (batch b, head-group g): 4 batches x 2 groups of 8 heads.
Each core computes QKV projections for its head slice, causal+padded softmax
attention, and a partial output projection; the host sums the two partial
outputs per batch and adds o_b (+ the V-bias contribution, which is constant
across queries because attention weights sum to 1).

Engine plan (per core):
  TensorE: projections (fp32r), scores kT.T@qT transposed [k, q], AV with
    augmented-V (ones column -> softmax denominators accumulate in PSUM row
    64), output projection in bf16 (FWL).
  ACT: exclusively exp (padding mask as per-partition bias).
  DVE: projection drains (+q bias), causal tri-mask on bf16 et, softmax
    normalize muls, PSUM->SBUF copies.
  Pool: denominator row broadcast. DMA: den row partition move, all loads.
Attention is software-pipelined per head (scores of chunk c+1 issue before
AV of chunk c so TensorE never idles on ACT); half-1 projections and w0
output projections are interleaved into the attention instruction streams.
K-bias is dropped (softmax shift invariance); V-bias folded into host o_b.
"""
import os
import sys

for _p in ("/opt/trn_rl_repo",):
    if _p not in sys.path:
        sys.path.insert(0, _p)

import numpy as np

HID = 1024
HEADS = 16
D = 64
BS = 4
SEQ = 2048
NCORES = 8
HG = 2             # head groups (tensor-parallel axis)
HPG = HEADS // HG  # 8 heads per core
OG = HPG * D       # 512 projection dims per core
KC = HID // 128    # 8 hidden chunks
SC = SEQ // 128    # 16 seq chunks
W = 1024           # attention query window
SCALE = 1.0 / np.sqrt(D)
NEG = -30000.0

_compiled = None


def _build():
    import concourse.tile as tile
    from concourse import bacc, mybir

    F32 = mybir.dt.float32
    F32R = mybir.dt.float32r
    BF16 = mybir.dt.bfloat16
    AF = mybir.ActivationFunctionType
    Alu = mybir.AluOpType

    nc = bacc.Bacc("TRN2", target_bir_lowering=False, debug=False,
                   num_devices=NCORES)

    xT_d = nc.dram_tensor("xT", [HID, SEQ], F32R, kind="ExternalInput").ap()
    wqT_d = nc.dram_tensor("wqT", [HID, OG], F32R, kind="ExternalInput").ap()
    wkT_d = nc.dram_tensor("wkT", [HID, OG], F32R, kind="ExternalInput").ap()
    wvT_d = nc.dram_tensor("wvT", [HID, OG], F32R, kind="ExternalInput").ap()
    woT_d = nc.dram_tensor("woT", [OG, HID], BF16, kind="ExternalInput").ap()
    qb_d = nc.dram_tensor("qb", [128, 4], F32, kind="ExternalInput").ap()
    kmask_d = nc.dram_tensor("kmask", [128, SC], F32, kind="ExternalInput").ap()
    out_d = nc.dram_tensor("out", [SEQ, HID], F32, kind="ExternalOutput").ap()

    with tile.TileContext(nc) as tc:
        with tc.tile_pool(name="const", bufs=1) as cp, \
             tc.tile_pool(name="qT", bufs=1) as qTp, \
             tc.tile_pool(name="kT", bufs=1) as kTp, \
             tc.tile_pool(name="v", bufs=1) as vp, \
             tc.tile_pool(name="attnT", bufs=1) as aTp, \
             tc.tile_pool(name="et", bufs=1) as etp, \
             tc.tile_pool(name="raw", bufs=1) as rawp, \
             tc.tile_pool(name="nrm", bufs=1) as nrmp, \
             tc.tile_pool(name="pp", bufs=1, space="PSUM") as ppp, \
             tc.tile_pool(name="sp", bufs=1, space="PSUM") as spp, \
             tc.tile_pool(name="at", bufs=1, space="PSUM") as atp:

            # ---------------- constants ----------------
            ones_f = cp.tile([128, 128], F32, tag="ones_f", name="ones_f")
            nc.gpsimd.memset(ones_f[:, :], 1.0)
            # tri01[p, j] = 1 if j >= p else 0  (keep keys <= query)
            tri01_f = cp.tile([128, 128], F32, tag="tri01_f", name="tri01_f")
            nc.gpsimd.affine_select(tri01_f[:, :], ones_f[:, :],
                                    pattern=[[1, 128]],
                                    compare_op=Alu.is_ge, fill=0.0,
                                    base=0, channel_multiplier=-1)
            tri01 = cp.tile([128, 128], BF16, tag="tri01", name="tri01")
            nc.scalar.copy(tri01[:, :], tri01_f[:, :])
            qb_s = cp.tile([128, 4], F32, tag="qb", name="qb_s")
            nc.sync.dma_start(qb_s[:, :], qb_d[:, :])
            kmask_s = cp.tile([128, SC], F32, tag="km", name="kmask_s")
            nc.sync.dma_start(kmask_s[:, :], kmask_d[:, :])

            # ---------------- persistent tensors ----------------
            qT_t = [qTp.tile([128, SEQ], F32R, tag=f"qT{i}", name=f"qT{i}")
                    for i in range(4)]
            kT_t = [kTp.tile([128, SEQ], F32R, tag=f"kT{i}", name=f"kT{i}")
                    for i in range(4)]
            v_t = [vp.tile([128, HPG * 65], BF16, tag=f"v{i}", name=f"v{i}")
                   for i in range(SC)]
            for i in range(SC):
                vv = v_t[i].rearrange("p (h c) -> p h c", c=65)
                nc.gpsimd.memset(vv[:, :, 64:65], 1.0)
            attnT_t = [aTp.tile([128, SEQ], BF16, tag=f"aT{i}", name=f"aT{i}")
                       for i in range(4)]
            den0 = nrmp.tile([1, W], F32, tag="den0", name="den0")
            denr = nrmp.tile([1, W], F32, tag="denr", name="denr")
            div = nrmp.tile([64, W], F32, tag="div", name="div")

            # ---------------- attention emission ----------------
            def attn_head(h, tw, fillers, rate):
                hb = (h % 2) * 64
                hc = h // 2
                chunks = [(c, 0, W) for c in range(8 * tw)]
                chunks += [(8 * tw + i, 128 * i, W - 128 * i) for i in range(8)]
                first = [None, None]
                last = [None, None]
                for idx, (c, off, w) in enumerate(chunks):
                    for half in range(2):
                        if max(off, half * 512) < (half + 1) * 512:
                            if first[half] is None:
                                first[half] = idx
                            last[half] = idx
                at = atp.tile([65, W], F32, tag="at", bufs=1, name="at")

                def escore(ch):
                    idx, (c, off, w) = ch
                    sp = spp.tile([128, W], F32, tag="sp", bufs=2, name="sp")
                    for half in range(2):
                        lo = max(off, half * 512)
                        hi = (half + 1) * 512
                        if lo >= hi:
                            continue
                        nc.tensor.matmul(
                            sp[:, lo:hi],
                            kT_t[hc][hb:hb + 64, c * 128:(c + 1) * 128],
                            qT_t[hc][hb:hb + 64, tw * W + lo:tw * W + hi],
                            start=True, stop=True)
                    return sp

                def eav(ch, sp):
                    idx, (c, off, w) = ch
                    et = etp.tile([128, W], BF16, tag="et", bufs=2, name="et")
                    nc.scalar.activation(et[:, :w], sp[:, off:off + w],
                                         AF.Exp,
                                         bias=kmask_s[:, c:c + 1],
                                         scale=SCALE)
                    if off or c == 8 * tw:  # diagonal chunk
                        nc.vector.tensor_mul(et[:, 0:128], et[:, 0:128],
                                             tri01[:, :])
                    for half in range(2):
                        lo = max(off, half * 512)
                        hi = (half + 1) * 512
                        if lo >= hi:
                            continue
                        nc.tensor.matmul(
                            at[0:65, lo:hi],
                            v_t[c][:, h * 65:(h + 1) * 65],
                            et[:, lo - off:hi - off],
                            start=(idx == first[half]),
                            stop=(idx == last[half]))

                prev = None
                for idx, ck in enumerate(chunks):
                    sp = escore((idx, ck))
                    if prev is not None:
                        eav(*prev)
                    prev = ((idx, ck), sp)
                    for _ in range(rate):
                        if fillers:
                            fillers.pop(0)()
                eav(*prev)

                # lazy normalize: copy out of PSUM fast, then recip+bcast+mul
                rawat = rawp.tile([65, W], F32, tag="raw", bufs=2, name="raw")
                nc.vector.tensor_copy(rawat[0:65, :], at[0:65, :])
                nc.sync.dma_start(den0[0:1, :], rawat[64:65, :])
                nc.vector.reciprocal_approx_fast(denr[0:1, :], den0[0:1, :])
                nc.gpsimd.partition_broadcast(div[0:64, :], denr[0:1, :])
                nc.vector.tensor_mul(
                    attnT_t[hc][hb:hb + 64, tw * W:(tw + 1) * W],
                    rawat[0:64, :], div[0:64, :])

            # ---------------- phase 1 + window 0 ----------------
            with tc.tile_pool(name="wqk", bufs=1) as wp, \
                 tc.tile_pool(name="wv", bufs=1) as wvp, \
                 tc.tile_pool(name="x", bufs=1) as xp, \
                 tc.tile_pool(name="xv", bufs=1) as xvp:

                wq_t, wk_t, wv_t = [], [], []
                for kc in range(KC):
                    wq = wp.tile([128, OG], F32R, tag=f"wq{kc}",
                                 name=f"wq{kc}")
                    nc.sync.dma_start(wq[:, :], wqT_d[kc * 128:(kc + 1) * 128, :])
                    wq_t.append(wq)
                for kc in range(KC):
                    wk = wp.tile([128, OG], F32R, tag=f"wk{kc}",
                                 name=f"wk{kc}")
                    nc.sync.dma_start(wk[:, :], wkT_d[kc * 128:(kc + 1) * 128, :])
                    wk_t.append(wk)
                for kc in range(KC):
                    wv = wvp.tile([128, OG], F32R, tag=f"wv{kc}",
                                  name=f"wv{kc}")
                    nc.sync.dma_start(wv[:, :], wvT_d[kc * 128:(kc + 1) * 128, :])
                    wv_t.append(wv)

                def x_group_load(half, g):
                    """Load x tiles for hidden chunks g*4..g*4+3 of seq half."""
                    tiles = []
                    for i in range(4):
                        kc = g * 4 + i
                        xt = xp.tile([128, W], F32R, tag=f"x{i}", bufs=1,
                                     name=f"x{half}{g}{i}")
                        nc.sync.dma_start(
                            xt[:, :], xT_d[kc * 128:(kc + 1) * 128,
                                           half * W:(half + 1) * W])
                        tiles.append(xt)
                    return tiles

                def qk_unit(wt, oc, g, xg, half, is_q):
                    """One (weight, out-chunk) slab over hidden group g."""
                    cbs = []
                    st = {}

                    def alloc():
                        st["p0"] = ppp.tile([128, 512], F32, tag="pp", bufs=2,
                                            name="p0")
                        st["p1"] = ppp.tile([128, 512], F32, tag="pp", bufs=2,
                                            name="p1")
                    cbs.append(alloc)
                    for j in range(4):
                        kc = g * 4 + j

                        def mm(j=j, kc=kc):
                            for t, pt in ((0, st["p0"]), (1, st["p1"])):
                                nc.tensor.matmul(
                                    pt[:, :],
                                    wt[kc][:, oc * 128:(oc + 1) * 128],
                                    xg[j][:, t * 512:(t + 1) * 512],
                                    start=(j == 0), stop=(j == 3))
                        cbs.append(mm)
                    o_t = qT_t if is_q else kT_t

                    def drain():
                        for t, pt in ((0, st["p0"]), (1, st["p1"])):
                            cols = slice(half * W + t * 512,
                                         half * W + t * 512 + 512)
                            if g == 0:
                                nc.vector.tensor_copy(o_t[oc][:, cols],
                                                      pt[:, :])
                            elif is_q:
                                nc.vector.scalar_tensor_tensor(
                                    o_t[oc][:, cols], pt[:, :],
                                    qb_s[:, oc:oc + 1], o_t[oc][:, cols],
                                    Alu.add, Alu.add)
                            else:
                                nc.vector.tensor_add(o_t[oc][:, cols],
                                                     o_t[oc][:, cols],
                                                     pt[:, :])
                    cbs.append(drain)
                    return cbs

                def v_unit(sc):
                    """V projection for seq chunk sc; x streamed per block."""
                    cbs = []
                    st = {"xv": [None] * KC}
                    for kc in range(KC):
                        def dma(kc=kc):
                            xv = xvp.tile([128, 128], F32R, tag=f"xv{kc}",
                                          bufs=2, name=f"xv{kc}")
                            nc.sync.dma_start(
                                xv[:, :], xT_d[kc * 128:(kc + 1) * 128,
                                               sc * 128:(sc + 1) * 128])
                            st["xv"][kc] = xv
                        cbs.append(dma)

                    def alloc():
                        st["pv"] = ppp.tile([128, 512], F32, tag="pp", bufs=2,
                                            name="pv")
                    cbs.append(alloc)
                    for kc in range(KC):
                        def mm(kc=kc):
                            nc.tensor.matmul(st["pv"][:, :], st["xv"][kc][:, :],
                                             wv_t[kc][:, :],
                                             start=(kc == 0),
                                             stop=(kc == KC - 1))
                        cbs.append(mm)

                    def drain():
                        src = st["pv"].rearrange("p (h c) -> p h c", c=64)
                        dst = v_t[sc].rearrange("p (h c) -> p h c", c=65)
                        nc.vector.tensor_copy(dst[:, :, 0:64], src[:, :, :])
                    cbs.append(drain)
                    return cbs

                # --- phase 1a: seq half 0, serial ---
                xg = x_group_load(0, 0)
                for cb in [c for oc in range(4)
                           for c in qk_unit(wq_t, oc, 0, xg, 0, True)]:
                    cb()
                for cb in [c for oc in range(4)
                           for c in qk_unit(wk_t, oc, 0, xg, 0, False)]:
                    cb()
                xg = x_group_load(0, 1)
                for cb in [c for oc in range(4)
                           for c in qk_unit(wq_t, oc, 1, xg, 0, True)]:
                    cb()
                for cb in [c for oc in range(4)
                           for c in qk_unit(wk_t, oc, 1, xg, 0, False)]:
                    cb()
                for sc in range(8):
                    for cb in v_unit(sc):
                        cb()

                # --- window 0 attention + interleaved half-1 projections ---
                fillers = []
                xg1 = {"g0": None, "g1": None}
                def ld_g0():
                    xg1["g0"] = x_group_load(1, 0)
                fillers.append(ld_g0)
                for oc in range(4):
                    fillers += qk_unit(wq_t, oc, 0,
                                       _lazy(xg1, "g0"), 1, True)
                for oc in range(4):
                    fillers += qk_unit(wk_t, oc, 0,
                                       _lazy(xg1, "g0"), 1, False)
                def ld_g1():
                    xg1["g1"] = x_group_load(1, 1)
                fillers.append(ld_g1)
                for oc in range(4):
                    fillers += qk_unit(wq_t, oc, 1,
                                       _lazy(xg1, "g1"), 1, True)
                for oc in range(4):
                    fillers += qk_unit(wk_t, oc, 1,
                                       _lazy(xg1, "g1"), 1, False)
                for sc in range(8, SC):
                    fillers += v_unit(sc)

                for h in range(HPG):
                    attn_head(h, 0, fillers, rate=0 if h == 0 else 7)
                while fillers:
                    fillers.pop(0)()

            # ---------------- window 1 + w0 output projection ----------------
            with tc.tile_pool(name="wo", bufs=1) as wop, \
                 tc.tile_pool(name="ot", bufs=1) as otp:

                wo_t = [None] * 4

                def oproj_unit(sc):
                    cbs = []
                    st = {}

                    def alloc():
                        st["ot"] = otp.tile([128, HID], F32, tag="ot", bufs=2,
                                            name="ot")
                    cbs.append(alloc)
                    for n in range(2):
                        def palloc(n=n):
                            st["po"] = ppp.tile([128, 512], F32, tag="pp",
                                                bufs=2, name="po")
                        cbs.append(palloc)
                        for kc in range(4):
                            def mm(n=n, kc=kc):
                                nc.tensor.matmul(
                                    st["po"][:, :],
                                    attnT_t[kc][:, sc * 128:(sc + 1) * 128],
                                    wo_t[kc][:, n * 512:(n + 1) * 512],
                                    start=(kc == 0), stop=(kc == 3))
                            cbs.append(mm)

                        def drain(n=n):
                            nc.vector.tensor_copy(
                                st["ot"][:, n * 512:(n + 1) * 512],
                                st["po"][:, :])
                        cbs.append(drain)

                    def store():
                        nc.sync.dma_start(out_d[sc * 128:(sc + 1) * 128, :],
                                          st["ot"][:, :])
                    cbs.append(store)
                    return cbs

                fillers = []
                def ld_wo():
                    for kc in range(4):
                        wo = wop.tile([128, HID], BF16, tag=f"wo{kc}",
                                      name=f"wo{kc}")
                        nc.sync.dma_start(wo[:, :],
                                          woT_d[kc * 128:(kc + 1) * 128, :])
                        wo_t[kc] = wo
                fillers.append(ld_wo)
                for sc in range(8):
                    fillers += oproj_unit(sc)

                for h in range(HPG):
                    attn_head(h, 1, fillers, rate=0 if h == 0 else 2)
                while fillers:
                    fillers.pop(0)()
                for sc in range(8, SC):
                    for cb in oproj_unit(sc):
                        cb()

    nc.compile()
    return nc


def _lazy(d, k):
    """Indexable proxy resolving d[k] at emission time (filler x tiles)."""
    class _P:
        def __getitem__(self, i):
            return d[k][i]
    return _P()


def kernel(hidden_states, causal_mask, padding_mask,
           q_w, q_b, k_w, k_b, v_w, v_b, o_w, o_b):
    global _compiled
    import ml_dtypes
    from concourse.bass_utils import run_bass_kernel_spmd

    hidden_states = np.asarray(hidden_states, dtype=np.float32)
    padding_mask = np.asarray(padding_mask)
    q_w = np.asarray(q_w, dtype=np.float32)
    k_w = np.asarray(k_w, dtype=np.float32)
    v_w = np.asarray(v_w, dtype=np.float32)
    o_w = np.asarray(o_w, dtype=np.float32)
    q_b = np.asarray(q_b, dtype=np.float32)
    v_b = np.asarray(v_b, dtype=np.float32)
    o_b = np.asarray(o_b, dtype=np.float32)

    if _compiled is None:
        _compiled = _build()
    nc = _compiled

    in_maps = []
    for b in range(BS):
        xT = np.ascontiguousarray(hidden_states[b].T)
        kmask = np.where(padding_mask[b], np.float32(NEG),
                         np.float32(0.0)).astype(np.float32)
        kmask2 = np.ascontiguousarray(kmask.reshape(SC, 128).T)
        for g in range(HG):
            r = slice(g * OG, (g + 1) * OG)
            in_maps.append({
                "xT": xT,
                "wqT": np.ascontiguousarray(q_w[r].T),
                "wkT": np.ascontiguousarray(k_w[r].T),
                "wvT": np.ascontiguousarray(v_w[r].T),
                "woT": np.ascontiguousarray(o_w[:, r].T).astype(
                    ml_dtypes.bfloat16),
                "qb": np.ascontiguousarray(q_b[r].reshape(4, 128).T),
                "kmask": kmask2,
            })

    trace = os.environ.get("KERNEL_TRACE") == "1"
    res = run_bass_kernel_spmd(nc, in_maps, core_ids=list(range(NCORES)),
                               trace=trace)
    if trace and res.exec_time_ns is not None:
        print(f"HW exec time: {res.exec_time_ns} ns")
        if res.instructions_and_trace:
            print(f"trace: {res.instructions_and_trace[1]}")

    # host: sum head-group partials, add o_b and the V-bias contribution
    vb_term = o_w @ v_b  # [HID]; exact because attention weights sum to 1
    const = (o_b + vb_term)[None, :]
    out = np.empty((BS, SEQ, HID), dtype=np.float32)
    for b in range(BS):
        out[b] = (res.results[2 * b]["out"] + res.results[2 * b + 1]["out"]
                  + const)
    return out


# revision 4
# speedup vs baseline: 1.3475x; 1.0798x over previous
"""Multi-head attention (bs=4, seq=2048, hidden=1024, 16 heads) on 8 trn2 cores.

Sharding: core = (batch b, head-group g): 4 batches x 2 groups of 8 heads.
Each core computes QKV projections for its head slice, causal+padded softmax
attention, and a partial output projection; the host sums the two partial
outputs per batch and adds o_b (+ the V-bias contribution, constant across
queries because attention weights sum to 1). K-bias is dropped entirely
(softmax shift invariance).

Engine plan (per core):
  TensorE: QK projections (fp32r), V projection (bf16, FWL), scores
    kT.T@qT transposed [k, q] (fp32r), AV with augmented-V (ones column ->
    softmax denominators accumulate in PSUM row 64), output projection bf16.
  ACT: exclusively exp (padding mask as per-partition bias).
  DVE: projection drains (+q bias), causal tri-mask on bf16 et, softmax
    normalize muls, PSUM->SBUF copies.
  Pool: denominator row broadcast. DMA: den-row partition move, all loads.
Attention is software-pipelined per head (scores of chunk c+1 issue before
AV of chunk c so TensorE never idles on ACT); V1 projections fill window-0
attention, w0 output projections fill window-1 attention.
"""
import os
import sys

for _p in ("/opt/trn_rl_repo",):
    if _p not in sys.path:
        sys.path.insert(0, _p)

import numpy as np

HID = 1024
HEADS = 16
D = 64
BS = 4
SEQ = 2048
NCORES = 8
HG = 2             # head groups (tensor-parallel axis)
HPG = HEADS // HG  # 8 heads per core
OG = HPG * D       # 512 projection dims per core
KC = HID // 128    # 8 hidden chunks
SC = SEQ // 128    # 16 seq chunks
W = 1024           # attention query window
SCALE = 1.0 / np.sqrt(D)
NEG = -30000.0

_compiled = None


def _build():
    import concourse.tile as tile
    from concourse import bacc, mybir

    F32 = mybir.dt.float32
    F32R = mybir.dt.float32r
    BF16 = mybir.dt.bfloat16
    AF = mybir.ActivationFunctionType
    Alu = mybir.AluOpType

    nc = bacc.Bacc("TRN2", target_bir_lowering=False, debug=False,
                   num_devices=NCORES)

    xT_d = nc.dram_tensor("xT", [HID, SEQ], F32R, kind="ExternalInput").ap()
    xTb_d = nc.dram_tensor("xTb", [HID, SEQ], BF16, kind="ExternalInput").ap()
    wqT_d = nc.dram_tensor("wqT", [HID, OG], F32R, kind="ExternalInput").ap()
    wkT_d = nc.dram_tensor("wkT", [HID, OG], F32R, kind="ExternalInput").ap()
    wvT_d = nc.dram_tensor("wvT", [HID, OG], BF16, kind="ExternalInput").ap()
    woT_d = nc.dram_tensor("woT", [OG, HID], BF16, kind="ExternalInput").ap()
    qb_d = nc.dram_tensor("qb", [128, 4], F32, kind="ExternalInput").ap()
    kmask_d = nc.dram_tensor("kmask", [128, SC], F32, kind="ExternalInput").ap()
    out_d = nc.dram_tensor("out", [SEQ, HID], F32, kind="ExternalOutput").ap()

    with tile.TileContext(nc) as tc:
        with tc.tile_pool(name="const", bufs=1) as cp, \
             tc.tile_pool(name="qT", bufs=1) as qTp, \
             tc.tile_pool(name="kT", bufs=1) as kTp, \
             tc.tile_pool(name="v", bufs=1) as vp, \
             tc.tile_pool(name="attnT", bufs=1) as aTp, \
             tc.tile_pool(name="wv", bufs=1) as wvp, \
             tc.tile_pool(name="xv", bufs=1) as xvp:

            # ---------------- constants ----------------
            ones_f = cp.tile([128, 128], F32, tag="ones_f", name="ones_f")
            nc.gpsimd.memset(ones_f[:, :], 1.0)
            # tri01[p, j] = 1 if j >= p else 0  (keep keys <= query)
            tri01_f = cp.tile([128, 128], F32, tag="tri01_f", name="tri01_f")
            nc.gpsimd.affine_select(tri01_f[:, :], ones_f[:, :],
                                    pattern=[[1, 128]],
                                    compare_op=Alu.is_ge, fill=0.0,
                                    base=0, channel_multiplier=-1)
            tri01 = cp.tile([128, 128], BF16, tag="tri01", name="tri01")
            nc.scalar.copy(tri01[:, :], tri01_f[:, :])
            qb_s = cp.tile([128, 4], F32, tag="qb", name="qb_s")
            nc.sync.dma_start(qb_s[:, :], qb_d[:, :])
            kmask_s = cp.tile([128, SC], F32, tag="km", name="kmask_s")
            nc.sync.dma_start(kmask_s[:, :], kmask_d[:, :])

            # ---------------- persistent tensors ----------------
            qT_t = [qTp.tile([128, SEQ], F32R, tag=f"qT{i}", name=f"qT{i}")
                    for i in range(4)]
            kT_t = [kTp.tile([128, SEQ], F32R, tag=f"kT{i}", name=f"kT{i}")
                    for i in range(4)]
            v_t = [vp.tile([128, HPG * 65], BF16, tag=f"v{i}", name=f"v{i}")
                   for i in range(SC)]
            for i in range(SC):
                vv = v_t[i].rearrange("p (h c) -> p h c", c=65)
                nc.gpsimd.memset(vv[:, :, 64:65], 1.0)
            attnT_t = [aTp.tile([128, SEQ], BF16, tag=f"aT{i}", name=f"aT{i}")
                       for i in range(4)]

            # =========== region 1: QK projections (all seq) + V ===========
            with tc.tile_pool(name="wqk", bufs=1) as wp, \
                 tc.tile_pool(name="x", bufs=1) as xp, \
                 tc.tile_pool(name="phA", bufs=1, space="PSUM") as phA:

                wq_t, wk_t, wv_t = [], [], []
                for kc in range(KC):
                    wq = wp.tile([128, OG], F32R, tag=f"wq{kc}",
                                 name=f"wq{kc}")
                    nc.sync.dma_start(wq[:, :],
                                      wqT_d[kc * 128:(kc + 1) * 128, :])
                    wq_t.append(wq)
                for kc in range(KC):
                    wk = wp.tile([128, OG], F32R, tag=f"wk{kc}",
                                 name=f"wk{kc}")
                    nc.sync.dma_start(wk[:, :],
                                      wkT_d[kc * 128:(kc + 1) * 128, :])
                    wk_t.append(wk)
                for kc in range(KC):
                    wv = wvp.tile([128, OG], BF16, tag=f"wv{kc}",
                                  name=f"wv{kc}")
                    nc.sync.dma_start(wv[:, :],
                                      wvT_d[kc * 128:(kc + 1) * 128, :])
                    wv_t.append(wv)

                def x_group_load(half, g):
                    tiles = []
                    for i in range(4):
                        kc = g * 4 + i
                        xt = xp.tile([128, W], F32R, tag=f"x{i}", bufs=1,
                                     name=f"x{half}{g}{i}")
                        nc.sync.dma_start(
                            xt[:, :], xT_d[kc * 128:(kc + 1) * 128,
                                           half * W:(half + 1) * W])
                        tiles.append(xt)
                    return tiles

                def qk_unit(wt, oc, g, xg, half, is_q):
                    p0 = phA.tile([128, 512], F32, tag="pts", bufs=6,
                                  name="p0")
                    p1 = phA.tile([128, 512], F32, tag="pts", bufs=6,
                                  name="p1")
                    for j in range(4):
                        kc = g * 4 + j
                        for t, pt in ((0, p0), (1, p1)):
                            nc.tensor.matmul(
                                pt[:, :],
                                wt[kc][:, oc * 128:(oc + 1) * 128],
                                xg[j][:, t * 512:(t + 1) * 512],
                                start=(j == 0), stop=(j == 3))
                    o_t = qT_t if is_q else kT_t
                    for t, pt in ((0, p0), (1, p1)):
                        cols = slice(half * W + t * 512,
                                     half * W + t * 512 + 512)
                        if g == 0:
                            nc.vector.tensor_copy(o_t[oc][:, cols], pt[:, :])
                        elif is_q:
                            nc.vector.scalar_tensor_tensor(
                                o_t[oc][:, cols], pt[:, :],
                                qb_s[:, oc:oc + 1], o_t[oc][:, cols],
                                Alu.add, Alu.add)
                        else:
                            nc.vector.tensor_add(o_t[oc][:, cols],
                                                 o_t[oc][:, cols], pt[:, :])

                def v_unit(sc, pool_ref):
                    """V projection for seq chunk sc; bf16 x streamed.
                    pool_ref: 1-elem list holding (psum_pool, tag) at
                    emission time."""
                    cbs = []
                    st = {"xv": [None] * KC}
                    for kc in range(KC):
                        def dma(kc=kc):
                            xv = xvp.tile([128, 128], BF16, tag=f"xv{kc}",
                                          bufs=2, name=f"xv{kc}")
                            nc.sync.dma_start(
                                xv[:, :], xTb_d[kc * 128:(kc + 1) * 128,
                                                sc * 128:(sc + 1) * 128])
                            st["xv"][kc] = xv
                        cbs.append(dma)

                    def alloc():
                        pool, tag = pool_ref[0]
                        st["pv"] = pool.tile([128, 512], F32, tag=tag,
                                             bufs=2, name="pv")
                    cbs.append(alloc)
                    for kc in range(KC):
                        def mm(kc=kc):
                            nc.tensor.matmul(st["pv"][:, :],
                                             st["xv"][kc][:, :],
                                             wv_t[kc][:, :],
                                             start=(kc == 0),
                                             stop=(kc == KC - 1))
                        cbs.append(mm)

                    def drain():
                        src = st["pv"].rearrange("p (h c) -> p h c", c=64)
                        dst = v_t[sc].rearrange("p (h c) -> p h c", c=65)
                        nc.vector.tensor_copy(dst[:, :, 0:64], src[:, :, :])
                    cbs.append(drain)
                    return cbs

                xg = x_group_load(0, 0)
                for oc in range(4):
                    qk_unit(wq_t, oc, 0, xg, 0, True)
                for oc in range(4):
                    qk_unit(wk_t, oc, 0, xg, 0, False)
                xg = x_group_load(0, 1)
                for oc in range(4):
                    qk_unit(wq_t, oc, 1, xg, 0, True)
                for oc in range(4):
                    qk_unit(wk_t, oc, 1, xg, 0, False)
                # V for seq chunks 0..7 here; the x half-1 group loads hide
                # behind these V matmuls.
                xg = x_group_load(1, 0)
                for sc in range(8):
                    for cb in v_unit(sc, [(phA, "pv")]):
                        cb()
                for oc in range(4):
                    qk_unit(wq_t, oc, 0, xg, 1, True)
                for oc in range(4):
                    qk_unit(wk_t, oc, 0, xg, 1, False)
                xg = x_group_load(1, 1)
                for oc in range(4):
                    qk_unit(wq_t, oc, 1, xg, 1, True)
                for oc in range(4):
                    qk_unit(wk_t, oc, 1, xg, 1, False)

                v1_pool_ref = [None]
                v1_units = [v_unit(sc, v1_pool_ref) for sc in range(8, SC)]

            # ============ region 2: attention + output projection ============
            with tc.tile_pool(name="et", bufs=1) as etp, \
                 tc.tile_pool(name="raw", bufs=1) as rawp, \
                 tc.tile_pool(name="nrm", bufs=1) as nrmp, \
                 tc.tile_pool(name="wo", bufs=1) as wop, \
                 tc.tile_pool(name="ot", bufs=1) as otp, \
                 tc.tile_pool(name="sp", bufs=1, space="PSUM") as spp, \
                 tc.tile_pool(name="at", bufs=1, space="PSUM") as atp, \
                 tc.tile_pool(name="po", bufs=1, space="PSUM") as pop:

                den0 = nrmp.tile([1, W], F32, tag="den0", name="den0")
                denr = nrmp.tile([1, W], F32, tag="denr", name="denr")
                div = nrmp.tile([64, W], F32, tag="div", name="div")

                def attn_head(h, tw, fillers, rate):
                    hb = (h % 2) * 64
                    hc = h // 2
                    chunks = [(c, 0, W) for c in range(8 * tw)]
                    chunks += [(8 * tw + i, 128 * i, W - 128 * i)
                               for i in range(8)]
                    first = [None, None]
                    last = [None, None]
                    for idx, (c, off, w) in enumerate(chunks):
                        for half in range(2):
                            if max(off, half * 512) < (half + 1) * 512:
                                if first[half] is None:
                                    first[half] = idx
                                last[half] = idx
                    at = atp.tile([65, W], F32, tag="at", bufs=1, name="at")

                    def escore(idx, c, off, w):
                        sp = spp.tile([128, W], F32, tag="sp", bufs=2,
                                      name="sp")
                        for half in range(2):
                            lo = max(off, half * 512)
                            hi = (half + 1) * 512
                            if lo >= hi:
                                continue
                            nc.tensor.matmul(
                                sp[:, lo:hi],
                                kT_t[hc][hb:hb + 64, c * 128:(c + 1) * 128],
                                qT_t[hc][hb:hb + 64,
                                         tw * W + lo:tw * W + hi],
                                start=True, stop=True)
                        return sp

                    def eav(idx, c, off, w, sp):
                        et = etp.tile([128, W], BF16, tag="et", bufs=2,
                                      name="et")
                        nc.scalar.activation(et[:, :w], sp[:, off:off + w],
                                             AF.Exp,
                                             bias=kmask_s[:, c:c + 1],
                                             scale=SCALE)
                        if off or c == 8 * tw:  # diagonal chunk
                            nc.vector.tensor_mul(et[:, 0:128], et[:, 0:128],
                                                 tri01[:, :])
                        for half in range(2):
                            lo = max(off, half * 512)
                            hi = (half + 1) * 512
                            if lo >= hi:
                                continue
                            nc.tensor.matmul(
                                at[0:65, lo:hi],
                                v_t[c][:, h * 65:(h + 1) * 65],
                                et[:, lo - off:hi - off],
                                start=(idx == first[half]),
                                stop=(idx == last[half]))

                    prev = None
                    for idx, (c, off, w) in enumerate(chunks):
                        sp = escore(idx, c, off, w)
                        if prev is not None:
                            eav(*prev)
                        prev = (idx, c, off, w, sp)
                        for _ in range(rate):
                            if fillers:
                                fillers.pop(0)()
                    eav(*prev)

                    # lazy normalize: fast PSUM drain, then recip+bcast+mul
                    rawat = rawp.tile([65, W], F32, tag="raw", bufs=2,
                                      name="raw")
                    nc.vector.tensor_copy(rawat[0:65, :], at[0:65, :])
                    nc.sync.dma_start(den0[0:1, :], rawat[64:65, :])
                    nc.vector.reciprocal_approx_fast(denr[0:1, :],
                                                     den0[0:1, :])
                    nc.gpsimd.partition_broadcast(div[0:64, :], denr[0:1, :])
                    nc.vector.tensor_mul(
                        attnT_t[hc][hb:hb + 64, tw * W:(tw + 1) * W],
                        rawat[0:64, :], div[0:64, :])

                wo_t = [None] * 4

                def oproj_unit(sc):
                    cbs = []
                    st = {}

                    def alloc():
                        st["ot"] = otp.tile([128, HID], F32, tag="ot",
                                            bufs=2, name="ot")
                    cbs.append(alloc)
                    for n in range(2):
                        def palloc(n=n):
                            st["po"] = pop.tile([128, 512], F32, tag="po",
                                                bufs=2, name="po")
                        cbs.append(palloc)
                        for kc in range(4):
                            def mm(n=n, kc=kc):
                                nc.tensor.matmul(
                                    st["po"][:, :],
                                    attnT_t[kc][:, sc * 128:(sc + 1) * 128],
                                    wo_t[kc][:, n * 512:(n + 1) * 512],
                                    start=(kc == 0), stop=(kc == 3))
                            cbs.append(mm)

                        def drain(n=n):
                            nc.vector.tensor_copy(
                                st["ot"][:, n * 512:(n + 1) * 512],
                                st["po"][:, :])
                        cbs.append(drain)

                    def store():
                        nc.sync.dma_start(
                            out_d[sc * 128:(sc + 1) * 128, :], st["ot"][:, :])
                    cbs.append(store)
                    return cbs

                # window 0: V1 projections (sc 8..15) fill the gaps
                v1_pool_ref[0] = (pop, "po")
                fillers = []
                for cbs in v1_units:
                    fillers.extend(cbs)
                for h in range(HPG):
                    attn_head(h, 0, fillers, rate=0 if h == 0 else 4)
                while fillers:
                    fillers.pop(0)()

                # window 1: w0 output projections fill the gaps
                fillers = []
                def ld_wo():
                    for kc in range(4):
                        wo = wop.tile([128, HID], BF16, tag=f"wo{kc}",
                                      name=f"wo{kc}")
                        nc.sync.dma_start(
                            wo[:, :], woT_d[kc * 128:(kc + 1) * 128, :])
                        wo_t[kc] = wo
                fillers.append(ld_wo)
                for sc in range(8):
                    fillers += oproj_unit(sc)
                for h in range(HPG):
                    attn_head(h, 1, fillers, rate=0 if h == 0 else 2)
                while fillers:
                    fillers.pop(0)()
                for sc in range(8, SC):
                    for cb in oproj_unit(sc):
                        cb()

    nc.compile()
    return nc


def kernel(hidden_states, causal_mask, padding_mask,
           q_w, q_b, k_w, k_b, v_w, v_b, o_w, o_b):
    global _compiled
    import ml_dtypes
    from concourse.bass_utils import run_bass_kernel_spmd

    hidden_states = np.asarray(hidden_states, dtype=np.float32)
    padding_mask = np.asarray(padding_mask)
    q_w = np.asarray(q_w, dtype=np.float32)
    k_w = np.asarray(k_w, dtype=np.float32)
    v_w = np.asarray(v_w, dtype=np.float32)
    o_w = np.asarray(o_w, dtype=np.float32)
    q_b = np.asarray(q_b, dtype=np.float32)
    v_b = np.asarray(v_b, dtype=np.float32)
    o_b = np.asarray(o_b, dtype=np.float32)

    if _compiled is None:
        _compiled = _build()
    nc = _compiled

    in_maps = []
    for b in range(BS):
        xT = np.ascontiguousarray(hidden_states[b].T)
        xTb = xT.astype(ml_dtypes.bfloat16)
        kmask = np.where(padding_mask[b], np.float32(NEG),
                         np.float32(0.0)).astype(np.float32)
        kmask2 = np.ascontiguousarray(kmask.reshape(SC, 128).T)
        for g in range(HG):
            r = slice(g * OG, (g + 1) * OG)
            in_maps.append({
                "xT": xT,
                "xTb": xTb,
                "wqT": np.ascontiguousarray(q_w[r].T),
                "wkT": np.ascontiguousarray(k_w[r].T),
                "wvT": np.ascontiguousarray(v_w[r].T).astype(
                    ml_dtypes.bfloat16),
                "woT": np.ascontiguousarray(o_w[:, r].T).astype(
                    ml_dtypes.bfloat16),
                "qb": np.ascontiguousarray(q_b[r].reshape(4, 128).T),
                "kmask": kmask2,
            })

    trace = os.environ.get("KERNEL_TRACE") == "1"
    res = run_bass_kernel_spmd(nc, in_maps, core_ids=list(range(NCORES)),
                               trace=trace)
    if trace and res.exec_time_ns is not None:
        print(f"HW exec time: {res.exec_time_ns} ns")
        if res.instructions_and_trace:
            print(f"trace: {res.instructions_and_trace[1]}")

    # host: sum head-group partials, add o_b and the V-bias contribution
    vb_term = o_w @ v_b  # [HID]; exact because attention weights sum to 1
    const = (o_b + vb_term)[None, :]
    out = np.empty((BS, SEQ, HID), dtype=np.float32)
    for b in range(BS):
        out[b] = (res.results[2 * b]["out"] + res.results[2 * b + 1]["out"]
                  + const)
    return out


# revision 6
# speedup vs baseline: 1.5950x; 1.1837x over previous
"""Multi-head attention (bs=4, seq=2048, hidden=1024, 16 heads) on 8 trn2 cores.

Sharding: core = (batch b, head-group g): 4 batches x 2 groups of 8 heads.
Each core computes QKV projections for its head slice, causal+padded softmax
attention, and a partial output projection; the host sums the two partial
outputs per batch and adds o_b (+ the V-bias contribution, constant across
queries because attention weights sum to 1). K-bias is dropped entirely
(softmax shift invariance).

Engine plan (per core):
  TensorE: QK projections (fp32r), V projection (bf16, FWL), scores
    kT.T@qT transposed [k, q] (fp32r), AV with augmented-V (ones column ->
    softmax denominators accumulate in PSUM row 64), output projection bf16.
  ACT: exclusively exp (padding mask as per-partition bias).
  DVE: projection drains (+q bias), causal tri-mask on bf16 et, softmax
    normalize muls, PSUM->SBUF copies.
  Pool: denominator row broadcast. DMA: den-row partition move, all loads.
Attention is software-pipelined per head (scores of chunk c+1 issue before
AV of chunk c so TensorE never idles on ACT); V1 projections fill window-0
attention, w0 output projections fill window-1 attention.
"""
import os
import sys

for _p in ("/opt/trn_rl_repo",):
    if _p not in sys.path:
        sys.path.insert(0, _p)

import numpy as np

HID = 1024
HEADS = 16
D = 64
BS = 4
SEQ = 2048
NCORES = 8
HG = 2             # head groups (tensor-parallel axis)
HPG = HEADS // HG  # 8 heads per core
OG = HPG * D       # 512 projection dims per core
KC = HID // 128    # 8 hidden chunks
SC = SEQ // 128    # 16 seq chunks
W = 1024           # attention query window
SCALE = 1.0 / np.sqrt(D)
NEG = -30000.0

_compiled = None


def _build():
    import concourse.tile as tile
    from concourse import bacc, mybir

    F32 = mybir.dt.float32
    F32R = mybir.dt.float32r
    BF16 = mybir.dt.bfloat16
    AF = mybir.ActivationFunctionType
    Alu = mybir.AluOpType

    nc = bacc.Bacc("TRN2", target_bir_lowering=False, debug=False,
                   num_devices=NCORES)

    xT_d = nc.dram_tensor("xT", [HID, SEQ], F32R, kind="ExternalInput").ap()
    xTb_d = nc.dram_tensor("xTb", [HID, SEQ], BF16, kind="ExternalInput").ap()
    wqT_d = nc.dram_tensor("wqT", [HID, OG], F32R, kind="ExternalInput").ap()
    wkT_d = nc.dram_tensor("wkT", [HID, OG], F32R, kind="ExternalInput").ap()
    wvT_d = nc.dram_tensor("wvT", [HID, OG], BF16, kind="ExternalInput").ap()
    woT_d = nc.dram_tensor("woT", [OG, HID], BF16, kind="ExternalInput").ap()
    qb_d = nc.dram_tensor("qb", [128, 4], F32, kind="ExternalInput").ap()
    kmask_d = nc.dram_tensor("kmask", [128, SC], F32, kind="ExternalInput").ap()
    out_d = nc.dram_tensor("out", [SEQ, HID], F32, kind="ExternalOutput").ap()

    with tile.TileContext(nc) as tc:
        with tc.tile_pool(name="const", bufs=1) as cp, \
             tc.tile_pool(name="qT", bufs=1) as qTp, \
             tc.tile_pool(name="kT", bufs=1) as kTp, \
             tc.tile_pool(name="v", bufs=1) as vp, \
             tc.tile_pool(name="attnT", bufs=1) as aTp, \
             tc.tile_pool(name="wv", bufs=1) as wvp, \
             tc.tile_pool(name="xv", bufs=1) as xvp:

            # ---------------- constants ----------------
            ones_f = cp.tile([128, 128], F32, tag="ones_f", name="ones_f")
            nc.gpsimd.memset(ones_f[:, :], 1.0)
            # tri01[p, j] = 1 if j >= p else 0  (keep keys <= query)
            tri01_f = cp.tile([128, 128], F32, tag="tri01_f", name="tri01_f")
            nc.gpsimd.affine_select(tri01_f[:, :], ones_f[:, :],
                                    pattern=[[1, 128]],
                                    compare_op=Alu.is_ge, fill=0.0,
                                    base=0, channel_multiplier=-1)
            tri01 = cp.tile([128, 128], BF16, tag="tri01", name="tri01")
            nc.scalar.copy(tri01[:, :], tri01_f[:, :])
            qb_s = cp.tile([128, 4], F32, tag="qb", name="qb_s")
            nc.sync.dma_start(qb_s[:, :], qb_d[:, :])
            kmask_s = cp.tile([128, SC], F32, tag="km", name="kmask_s")
            nc.sync.dma_start(kmask_s[:, :], kmask_d[:, :])

            # ---------------- persistent tensors ----------------
            qT_t = [qTp.tile([128, SEQ], BF16, tag=f"qT{i}", name=f"qT{i}")
                    for i in range(4)]
            kT_t = [kTp.tile([128, SEQ], BF16, tag=f"kT{i}", name=f"kT{i}")
                    for i in range(4)]
            v_t = [vp.tile([128, HPG * 65], BF16, tag=f"v{i}", name=f"v{i}")
                   for i in range(SC)]
            for i in range(SC):
                vv = v_t[i].rearrange("p (h c) -> p h c", c=65)
                nc.gpsimd.memset(vv[:, :, 64:65], 1.0)
            attnT_t = [aTp.tile([128, SEQ], BF16, tag=f"aT{i}", name=f"aT{i}")
                       for i in range(4)]

            # =========== region 1: QK projections (all seq) + V ===========
            with tc.tile_pool(name="wqk", bufs=1) as wp, \
                 tc.tile_pool(name="x", bufs=1) as xp, \
                 tc.tile_pool(name="phA", bufs=1, space="PSUM") as phA:

                wq_t, wk_t, wv_t = [], [], []
                for kc in range(KC):
                    wq = wp.tile([128, OG], F32R, tag=f"wq{kc}",
                                 name=f"wq{kc}")
                    nc.sync.dma_start(wq[:, :],
                                      wqT_d[kc * 128:(kc + 1) * 128, :])
                    wq_t.append(wq)
                for kc in range(KC):
                    wk = wp.tile([128, OG], F32R, tag=f"wk{kc}",
                                 name=f"wk{kc}")
                    nc.sync.dma_start(wk[:, :],
                                      wkT_d[kc * 128:(kc + 1) * 128, :])
                    wk_t.append(wk)
                for kc in range(KC):
                    wv = wvp.tile([128, OG], BF16, tag=f"wv{kc}",
                                  name=f"wv{kc}")
                    nc.sync.dma_start(wv[:, :],
                                      wvT_d[kc * 128:(kc + 1) * 128, :])
                    wv_t.append(wv)

                def x_group_load(half, g):
                    tiles = []
                    for i in range(4):
                        kc = g * 4 + i
                        xt = xp.tile([128, W], F32R, tag=f"x{i}", bufs=1,
                                     name=f"x{half}{g}{i}")
                        nc.sync.dma_start(
                            xt[:, :], xT_d[kc * 128:(kc + 1) * 128,
                                           half * W:(half + 1) * W])
                        tiles.append(xt)
                    return tiles

                def qk_unit(wt, oc, g, xg, half, is_q):
                    p0 = phA.tile([128, 512], F32, tag="pts", bufs=6,
                                  name="p0")
                    p1 = phA.tile([128, 512], F32, tag="pts", bufs=6,
                                  name="p1")
                    for j in range(4):
                        kc = g * 4 + j
                        for t, pt in ((0, p0), (1, p1)):
                            nc.tensor.matmul(
                                pt[:, :],
                                wt[kc][:, oc * 128:(oc + 1) * 128],
                                xg[j][:, t * 512:(t + 1) * 512],
                                start=(j == 0), stop=(j == 3))
                    o_t = qT_t if is_q else kT_t
                    for t, pt in ((0, p0), (1, p1)):
                        cols = slice(half * W + t * 512,
                                     half * W + t * 512 + 512)
                        if g == 0:
                            nc.vector.tensor_copy(o_t[oc][:, cols], pt[:, :])
                        elif is_q:
                            nc.vector.scalar_tensor_tensor(
                                o_t[oc][:, cols], pt[:, :],
                                qb_s[:, oc:oc + 1], o_t[oc][:, cols],
                                Alu.add, Alu.add)
                        else:
                            nc.vector.tensor_add(o_t[oc][:, cols],
                                                 o_t[oc][:, cols], pt[:, :])

                def v_unit(sc, pool_ref):
                    """V projection for seq chunk sc; bf16 x streamed.
                    pool_ref: 1-elem list holding (psum_pool, tag) at
                    emission time."""
                    cbs = []
                    st = {"xv": [None] * KC}
                    for kc in range(KC):
                        def dma(kc=kc):
                            xv = xvp.tile([128, 128], BF16, tag=f"xv{kc}",
                                          bufs=2, name=f"xv{kc}")
                            nc.sync.dma_start(
                                xv[:, :], xTb_d[kc * 128:(kc + 1) * 128,
                                                sc * 128:(sc + 1) * 128])
                            st["xv"][kc] = xv
                        cbs.append(dma)

                    def alloc():
                        pool, tag = pool_ref[0]
                        st["pv"] = pool.tile([128, 512], F32, tag=tag,
                                             bufs=2, name="pv")
                    cbs.append(alloc)
                    for kc in range(KC):
                        def mm(kc=kc):
                            nc.tensor.matmul(st["pv"][:, :],
                                             st["xv"][kc][:, :],
                                             wv_t[kc][:, :],
                                             start=(kc == 0),
                                             stop=(kc == KC - 1))
                        cbs.append(mm)

                    def drain():
                        src = st["pv"].rearrange("p (h c) -> p h c", c=64)
                        dst = v_t[sc].rearrange("p (h c) -> p h c", c=65)
                        nc.vector.tensor_copy(dst[:, :, 0:64], src[:, :, :])
                    cbs.append(drain)
                    return cbs

                xg = x_group_load(0, 0)
                for oc in range(4):
                    qk_unit(wq_t, oc, 0, xg, 0, True)
                for oc in range(4):
                    qk_unit(wk_t, oc, 0, xg, 0, False)
                xg = x_group_load(0, 1)
                for oc in range(4):
                    qk_unit(wq_t, oc, 1, xg, 0, True)
                for oc in range(4):
                    qk_unit(wk_t, oc, 1, xg, 0, False)
                # V for seq chunks 0..7 here; the x half-1 group loads hide
                # behind these V matmuls.
                xg = x_group_load(1, 0)
                for sc in range(8):
                    for cb in v_unit(sc, [(phA, "pv")]):
                        cb()
                for oc in range(4):
                    qk_unit(wq_t, oc, 0, xg, 1, True)
                for oc in range(4):
                    qk_unit(wk_t, oc, 0, xg, 1, False)
                xg = x_group_load(1, 1)
                for oc in range(4):
                    qk_unit(wq_t, oc, 1, xg, 1, True)
                for oc in range(4):
                    qk_unit(wk_t, oc, 1, xg, 1, False)

                v1_pool_ref = [None]
                v1_units = [v_unit(sc, v1_pool_ref) for sc in range(8, SC)]

            # ============ region 2: attention + output projection ============
            with tc.tile_pool(name="et", bufs=1) as etp, \
                 tc.tile_pool(name="raw", bufs=1) as rawp, \
                 tc.tile_pool(name="nrm", bufs=1) as nrmp, \
                 tc.tile_pool(name="wo", bufs=1) as wop, \
                 tc.tile_pool(name="ot", bufs=1) as otp, \
                 tc.tile_pool(name="sp", bufs=1, space="PSUM") as spp, \
                 tc.tile_pool(name="at", bufs=1, space="PSUM") as atp, \
                 tc.tile_pool(name="po", bufs=1, space="PSUM") as pop:

                den0 = nrmp.tile([1, W], F32, tag="den0", name="den0")
                denr = nrmp.tile([1, W], F32, tag="denr", name="denr")
                div = nrmp.tile([64, W], F32, tag="div", name="div")

                def attn_head(h, tw, fillers, chunks_left):
                    hb = (h % 2) * 64
                    hc = h // 2
                    chunks = [(c, 0, W) for c in range(8 * tw)]
                    chunks += [(8 * tw + i, 128 * i, W - 128 * i)
                               for i in range(8)]
                    first = [None, None]
                    last = [None, None]
                    for idx, (c, off, w) in enumerate(chunks):
                        for half in range(2):
                            if max(off, half * 512) < (half + 1) * 512:
                                if first[half] is None:
                                    first[half] = idx
                                last[half] = idx
                    at = atp.tile([65, W], F32, tag="at", bufs=1, name="at")

                    def escore(idx, c, off, w):
                        sp = spp.tile([128, W], F32, tag="sp", bufs=2,
                                      name="sp")
                        for half in range(2):
                            lo = max(off, half * 512)
                            hi = (half + 1) * 512
                            if lo >= hi:
                                continue
                            nc.tensor.matmul(
                                sp[:, lo:hi],
                                kT_t[hc][hb:hb + 64, c * 128:(c + 1) * 128],
                                qT_t[hc][hb:hb + 64,
                                         tw * W + lo:tw * W + hi],
                                start=True, stop=True)
                        return sp

                    def eav(idx, c, off, w, sp):
                        et = etp.tile([128, W], BF16, tag="et", bufs=2,
                                      name="et")
                        nc.scalar.activation(et[:, :w], sp[:, off:off + w],
                                             AF.Exp,
                                             bias=kmask_s[:, c:c + 1],
                                             scale=SCALE)
                        if off or c == 8 * tw:  # diagonal chunk
                            nc.vector.tensor_mul(et[:, 0:128], et[:, 0:128],
                                                 tri01[:, :])
                        for half in range(2):
                            lo = max(off, half * 512)
                            hi = (half + 1) * 512
                            if lo >= hi:
                                continue
                            nc.tensor.matmul(
                                at[0:65, lo:hi],
                                v_t[c][:, h * 65:(h + 1) * 65],
                                et[:, lo - off:hi - off],
                                start=(idx == first[half]),
                                stop=(idx == last[half]))

                    prev = None
                    for idx, (c, off, w) in enumerate(chunks):
                        sp = escore(idx, c, off, w)
                        if prev is not None:
                            eav(*prev)
                        prev = (idx, c, off, w, sp)
                        n = -(-len(fillers) // max(chunks_left[0], 1))
                        for _ in range(min(n, 5)):
                            if fillers:
                                fillers.pop(0)()
                        chunks_left[0] -= 1
                    eav(*prev)

                    # lazy normalize: fast PSUM drain, then recip+bcast+mul
                    rawat = rawp.tile([65, W], F32, tag="raw", bufs=2,
                                      name="raw")
                    nc.vector.tensor_copy(rawat[0:65, :], at[0:65, :])
                    nc.sync.dma_start(den0[0:1, :], rawat[64:65, :])
                    nc.vector.reciprocal_approx_fast(denr[0:1, :],
                                                     den0[0:1, :])
                    nc.gpsimd.partition_broadcast(div[0:64, :], denr[0:1, :])
                    nc.vector.tensor_mul(
                        attnT_t[hc][hb:hb + 64, tw * W:(tw + 1) * W],
                        rawat[0:64, :], div[0:64, :])

                wo_t = [None] * 4

                def oproj_unit(sc):
                    cbs = []
                    st = {}

                    def alloc():
                        st["ot"] = otp.tile([128, HID], F32, tag="ot",
                                            bufs=2, name="ot")
                    cbs.append(alloc)
                    for n in range(2):
                        def palloc(n=n):
                            st["po"] = pop.tile([128, 512], F32, tag="po",
                                                bufs=2, name="po")
                        cbs.append(palloc)
                        for kc in range(4):
                            def mm(n=n, kc=kc):
                                nc.tensor.matmul(
                                    st["po"][:, :],
                                    attnT_t[kc][:, sc * 128:(sc + 1) * 128],
                                    wo_t[kc][:, n * 512:(n + 1) * 512],
                                    start=(kc == 0), stop=(kc == 3))
                            cbs.append(mm)

                        def drain(n=n):
                            nc.vector.tensor_copy(
                                st["ot"][:, n * 512:(n + 1) * 512],
                                st["po"][:, :])
                        cbs.append(drain)

                    def store():
                        nc.sync.dma_start(
                            out_d[sc * 128:(sc + 1) * 128, :], st["ot"][:, :])
                    cbs.append(store)
                    return cbs

                # window 0: V1 projections (sc 8..15) fill the gaps
                v1_pool_ref[0] = (pop, "po")
                fillers = []
                for cbs in v1_units:
                    fillers.extend(cbs)
                left = [8 * HPG]
                for h in range(HPG):
                    attn_head(h, 0, fillers, left)
                while fillers:
                    fillers.pop(0)()

                # window 1: w0 output projections fill the gaps
                fillers = []
                def ld_wo():
                    for kc in range(4):
                        wo = wop.tile([128, HID], BF16, tag=f"wo{kc}",
                                      name=f"wo{kc}")
                        nc.sync.dma_start(
                            wo[:, :], woT_d[kc * 128:(kc + 1) * 128, :])
                        wo_t[kc] = wo
                fillers.append(ld_wo)
                for sc in range(8):
                    fillers += oproj_unit(sc)
                left = [16 * HPG]
                for h in range(HPG):
                    attn_head(h, 1, fillers, left)
                while fillers:
                    fillers.pop(0)()
                for sc in range(8, SC):
                    for cb in oproj_unit(sc):
                        cb()

    nc.compile()
    return nc


def kernel(hidden_states, causal_mask, padding_mask,
           q_w, q_b, k_w, k_b, v_w, v_b, o_w, o_b):
    global _compiled
    import ml_dtypes
    from concourse.bass_utils import run_bass_kernel_spmd

    hidden_states = np.asarray(hidden_states, dtype=np.float32)
    padding_mask = np.asarray(padding_mask)
    q_w = np.asarray(q_w, dtype=np.float32)
    k_w = np.asarray(k_w, dtype=np.float32)
    v_w = np.asarray(v_w, dtype=np.float32)
    o_w = np.asarray(o_w, dtype=np.float32)
    q_b = np.asarray(q_b, dtype=np.float32)
    v_b = np.asarray(v_b, dtype=np.float32)
    o_b = np.asarray(o_b, dtype=np.float32)

    if _compiled is None:
        _compiled = _build()
    nc = _compiled

    in_maps = []
    for b in range(BS):
        xT = np.ascontiguousarray(hidden_states[b].T)
        xTb = xT.astype(ml_dtypes.bfloat16)
        kmask = np.where(padding_mask[b], np.float32(NEG),
                         np.float32(0.0)).astype(np.float32)
        kmask2 = np.ascontiguousarray(kmask.reshape(SC, 128).T)
        for g in range(HG):
            r = slice(g * OG, (g + 1) * OG)
            in_maps.append({
                "xT": xT,
                "xTb": xTb,
                "wqT": np.ascontiguousarray(q_w[r].T),
                "wkT": np.ascontiguousarray(k_w[r].T),
                "wvT": np.ascontiguousarray(v_w[r].T).astype(
                    ml_dtypes.bfloat16),
                "woT": np.ascontiguousarray(o_w[:, r].T).astype(
                    ml_dtypes.bfloat16),
                "qb": np.ascontiguousarray(q_b[r].reshape(4, 128).T),
                "kmask": kmask2,
            })

    trace = os.environ.get("KERNEL_TRACE") == "1"
    res = run_bass_kernel_spmd(nc, in_maps, core_ids=list(range(NCORES)),
                               trace=trace)
    if trace and res.exec_time_ns is not None:
        print(f"HW exec time: {res.exec_time_ns} ns")
        if res.instructions_and_trace:
            print(f"trace: {res.instructions_and_trace[1]}")

    # host: sum head-group partials, add o_b and the V-bias contribution
    vb_term = o_w @ v_b  # [HID]; exact because attention weights sum to 1
    const = (o_b + vb_term)[None, :]
    out = np.empty((BS, SEQ, HID), dtype=np.float32)
    for b in range(BS):
        out[b] = (res.results[2 * b]["out"] + res.results[2 * b + 1]["out"]
                  + const)
    return out
